# revision 46
# baseline (speedup 1.0000x reference)
"""Trainium2 Bass kernel for nn_MultiHeadAttention (B=8, S=1024, D=1024, h=16).

Sharding: pure data-parallel over batch - each of the 8 NeuronCores computes
the full MHA for one batch element. No collectives.

Host side pre-casts Q/K/V and the four weight matrices to bf16 (the PE
operands are bf16 regardless), halving input HBM traffic.

Schedule (v2): one software-pipelined stream built so that (a) the ScalarE
exp stream (~137us, the second-largest engine load) starts as early as
possible and never starves, and (b) the PE never idles >~3us (which would
trip the HAM clock-gate back to half rate).

  1. K projection runs first (relu evictions on the then-idle ScalarE) while
     the Q transposes stream in behind the K transposes on both HWDGE queues.
  2. Eight "units", one per head pair d: row-packed scores matmuls (even
     head on PE rows 0-63, odd head on rows 64-127, concurrent via
     tile_position auto-derive from base partitions) + exp ACTs, with PE
     filler pulled from a global deque between kb-steps: qproj(d+1) (relu
     eviction on DVE to keep ScalarE pure-exp), the whole V projection
     (during unit 0), and PV(d-1) + its softmax-division chain.
  3. Tail: PV(7) interleaved with the O-projection prologue (seq 0-2,
     contraction blocks 0-6; all earlier divisions already landed), then
     rolling finishes with evictions on the now-idle ScalarE.

PSUM: big ring 3 x [P,2,512] (6 banks) rotates kacc/qacc/spA/spB/oacc;
vp ring 2 x [P,512] (2 banks) rotates PV and vproj accumulators.

Attention core (unchanged from v1): scores kept transposed (keys on
partitions); exp(s/32) with no max-subtraction (scores are O(0.3)); per head
the PV stationary is 65 wide (64 V-features + a ones column accumulating the
softmax denominator); division via DMA-spread reciprocal + DMA broadcast +
one DVE multiply per half-head. Emission-order invariants (Tile derives
dependencies from emission order): vproj fully emitted before PV(0); PV(d)
emitted within unit d+1 early enough that its pt reads precede the exp
writes that recycle those pt ring slots (pt tiles are half-pairs, bufs=6).
"""
import os
from collections import deque
from contextlib import ExitStack

import ml_dtypes
import numpy as np

import concourse.bass as bass
import concourse.tile as tile
from concourse import mybir
from concourse.bass_utils import run_bass_kernel_spmd

f32 = mybir.dt.float32
bf16 = mybir.dt.bfloat16
AF = mybir.ActivationFunctionType

S = 1024
D = 1024
H = 16
DK = 64
P = 128
NB = D // P  # 8 blocks
QC = 512
N_CORES = 8


def _split_wide_waits(nc, max_waits=1):
    """This walrus build rejects instructions carrying more than one
    semaphore wait; move excess waits onto NoOp carriers inserted before
    the offending instruction on the same engine."""
    for bb in nc.m.functions[0].blocks:
        idx = 0
        while idx < len(bb.instructions):
            ins = bb.instructions[idx]
            si = ins.sync_info
            if si is not None and si.on_wait and len(si.on_wait) > max_waits:
                waits = list(si.on_wait)
                rest, keep = waits[:-max_waits], waits[-max_waits:]
                for j in range(0, len(rest), max_waits):
                    nop = mybir.InstNoOp(
                        name=f"I-waitsplit-{nc.next_id()}",
                        engine=ins.engine,
                        ins=[],
                        outs=[],
                    )
                    nop.sync_info = mybir.SyncInfo(
                        on_wait=rest[j : j + max_waits], on_update=[]
                    )
                    nc.register_instruction(nop)
                    bb.instructions.insert(idx, nop)
                    idx += 1
                ins.sync_info = mybir.SyncInfo(
                    on_wait=keep, on_update=list(si.on_update)
                )
            idx += 1


def _build_nc(with_bqk: bool, with_bv: bool, with_bo: bool):
    nc = bass.Bass("TRN2", target_bir_lowering=False, debug=False, num_devices=1)

    Qd = nc.dram_tensor("Q", [S, D], bf16, kind="ExternalInput").ap()
    Kd = nc.dram_tensor("K", [S, D], bf16, kind="ExternalInput").ap()
    Vd = nc.dram_tensor("V", [S, D], bf16, kind="ExternalInput").ap()
    WQd = nc.dram_tensor("WQ", [D, D], bf16, kind="ExternalInput").ap()
    WKd = nc.dram_tensor("WK", [D, D], bf16, kind="ExternalInput").ap()
    WVd = nc.dram_tensor("WV", [D, D], bf16, kind="ExternalInput").ap()
    WOd = nc.dram_tensor("WO", [D, D], bf16, kind="ExternalInput").ap()
    bQd = nc.dram_tensor("bQ", [D], f32, kind="ExternalInput").ap()
    bKd = nc.dram_tensor("bK", [D], f32, kind="ExternalInput").ap()
    bVd = nc.dram_tensor("bV", [D], f32, kind="ExternalInput").ap()
    bOd = nc.dram_tensor("bO", [D], f32, kind="ExternalInput").ap()
    outd = nc.dram_tensor("out", [S, D], f32, kind="ExternalOutput").ap()

    with tile.TileContext(nc) as tc, ExitStack() as ctx:
        sb = ctx.enter_context(tc.tile_pool(name="sb", bufs=1))
        ps = ctx.enter_context(tc.tile_pool(name="ps", bufs=1, space="PSUM"))
        dramp = ctx.enter_context(tc.tile_pool(name="dram", bufs=1, space="DRAM"))
        # softmax divisor path runs in bf16 (denominators are O(300) and
        # the correctness gate is 2e-2 rel; bf16 costs ~4e-4) - SBUF budget
        ctx.enter_context(nc.allow_low_precision(reason="bf16 softmax divisor"))

        # ---- constants -------------------------------------------------
        onesb = sb.tile([1, P], bf16, tag="onesb", name="onesb")
        nc.vector.memset(onesb, 1.0)

        def wload(Wd, eng):
            """One 2MB DMA: whole weight matrix as [p, kb, m] with
            wb[p, kb, m] = W[kb*128 + p, m]."""
            wb = sb.tile([P, NB, D], bf16, tag="wb", bufs=3, name="wb")
            eng.dma_start(wb, Wd.rearrange("(kb p) m -> p kb m", p=P))
            return wb

        def load_transposed(Xd, eng_pair):
            """HBM row-major bf16 -> feature-major tiles xt[db] (128 x 1024)
            via the DMA XBAR transpose, split across two HWDGE queues."""
            xt = [
                sb.tile([P, S], bf16, tag="xt", bufs=16, name=f"xt{i}")
                for i in range(NB)
            ]
            for db in range(NB):
                eng = eng_pair[db % 2]
                eng.dma_start_transpose(xt[db], Xd[:, db * P : (db + 1) * P])
            return xt

        # DMA issue order. K first (K projection runs first), Q right
        # behind on the same queues. V transposes + WO are emitted AFTER
        # the kproj matmuls (they recycle K xt / WK ring slots, and Tile
        # derives WAR deps from emission order), on the sync queue where
        # nothing urgent sits behind them until the unit-1 division DMAs.
        wk = wload(WKd, nc.scalar)
        kt = load_transposed(Kd, (nc.sync, nc.scalar))
        qt = load_transposed(Qd, (nc.sync, nc.scalar))
        wq = wload(WQd, nc.scalar)
        wv = wload(WVd, nc.sync)
        vt = []
        wo_h = []

        bqk = bv_row = bo_row = None
        if with_bqk:
            bqk = sb.tile([P, 2 * NB], f32, tag="bqk", name="bqk")
            nc.gpsimd.dma_start(bqk[:, 0:NB], bQd.rearrange("(db p) -> p db", p=P))
            nc.gpsimd.dma_start(
                bqk[:, NB : 2 * NB], bKd.rearrange("(db p) -> p db", p=P)
            )
        if with_bv:
            bvs = sb.tile([1, D], f32, tag="brows", bufs=2, name="bvs")
            nc.gpsimd.dma_start(bvs, bVd[None, :])
            bv_row = sb.tile([1, D], bf16, tag="browb", bufs=2, name="bv_row")
            nc.vector.tensor_copy(bv_row, bvs)
        if with_bo:
            bos = sb.tile([1, D], f32, tag="brows", bufs=2, name="bos")
            nc.gpsimd.dma_start(bos, bOd[None, :])
            bo_row = sb.tile([1, D], bf16, tag="browb", bufs=2, name="bo_row")
            nc.vector.tensor_copy(bo_row, bos)

        vaug = [
            sb.tile([P, H * 65], bf16, tag="vaug", bufs=NB, name=f"vaug{i}")
            for i in range(NB)
        ]
        for sblk in range(NB):
            nc.vector.memset(
                vaug[sblk].rearrange("p (h c) -> p h c", c=65)[:, :, 64:65], 1.0
            )

        # ---- projections -----------------------------------------------
        kpt = [
            sb.tile([P, S], bf16, tag="kpt", bufs=NB, name=f"kpt{i}")
            for i in range(NB)
        ]
        qpt_slot = {}

        def proj_evict(acc, dst, bias_idx, on_scalar):
            dst2 = dst.rearrange("p (c q) -> p c q", c=2)
            if with_bqk:
                nc.scalar.activation(
                    dst2, acc, AF.Relu, bias=bqk[:, bias_idx : bias_idx + 1]
                )
            elif on_scalar:
                nc.scalar.activation(dst2, acc, AF.Relu)
            else:
                nc.vector.tensor_scalar_max(dst2, acc, 0.0)

        def kproj_block(db):
            acc = ps.tile([P, 2, QC], f32, tag="big", bufs=3, name="kacc")
            co = db * P
            for kb in range(NB):
                wt = wk[:, kb, co : co + P]
                first, last = kb == 0, kb == NB - 1
                nc.tensor.matmul(
                    acc[:, 0, :], wt, kt[kb][:, 0:QC], start=first, stop=last
                )
                nc.tensor.matmul(
                    acc[:, 1, :], wt, kt[kb][:, QC:S], start=first, stop=last
                )
            proj_evict(acc, kpt[db], NB + db, on_scalar=True)

        def gen_kproj(db):
            acc = ps.tile([P, 2, QC], f32, tag="big", bufs=3, name="kacc")
            co = db * P
            for half in range(2):
                for kb in range(half * 4, half * 4 + 4):
                    wt = wk[:, kb, co : co + P]
                    first, last = kb == 0, kb == NB - 1
                    nc.tensor.matmul(
                        acc[:, 0, :], wt, kt[kb][:, 0:QC], start=first, stop=last
                    )
                    nc.tensor.matmul(
                        acc[:, 1, :], wt, kt[kb][:, QC:S], start=first, stop=last
                    )
                yield
            proj_evict(acc, kpt[db], NB + db, on_scalar=False)
            yield

        def gen_qproj(d, on_scalar=False):
            dst = sb.tile([P, S], bf16, tag="qpt", bufs=2, name=f"qpt{d}")
            qpt_slot[d] = dst
            acc = ps.tile([P, 2, QC], f32, tag="big", bufs=3, name="qacc")
            co = d * P
            for half in range(2):
                for kb in range(half * 4, half * 4 + 4):
                    wt = wq[:, kb, co : co + P]
                    first, last = kb == 0, kb == NB - 1
                    nc.tensor.matmul(
                        acc[:, 0, :], wt, qt[kb][:, 0:QC], start=first, stop=last
                    )
                    nc.tensor.matmul(
                        acc[:, 1, :], wt, qt[kb][:, QC:S], start=first, stop=last
                    )
                yield
            proj_evict(acc, dst, d, on_scalar=on_scalar)
            yield

        def gen_vproj(sblk):
            """V projection for one seq(key)-block -> vaug[sblk]."""
            acc = [
                ps.tile([P, QC], f32, tag="vp", bufs=2, name="vacc")
                for _ in range(2)
            ]
            if with_bv:
                for c in range(2):
                    nc.tensor.matmul(
                        acc[c], onesb[0:1, 0:P],
                        bv_row[0:1, c * QC : (c + 1) * QC],
                        start=True, stop=False,
                    )
            for half in range(2):
                for kb in range(half * 4, half * 4 + 4):
                    for c in range(2):
                        nc.tensor.matmul(
                            acc[c],
                            vt[kb][:, sblk * P : (sblk + 1) * P],
                            wv[:, kb, c * QC : (c + 1) * QC],
                            start=(kb == 0 and not with_bv),
                            stop=(kb == NB - 1),
                        )
                yield
            for c in range(2):
                dst = vaug[sblk].rearrange("p (h c) -> p h c", c=65)[
                    :, c * 8 : (c + 1) * 8, 0:64
                ]
                nc.vector.tensor_scalar_max(
                    dst, acc[c].rearrange("p (h c) -> p h c", c=64), 0.0
                )
            yield

        # ---- softmax division ------------------------------------------
        ot = [
            sb.tile([P, S], bf16, tag="ot", bufs=NB, name=f"ot{i}")
            for i in range(NB)
        ]
        div_pend = []

        def div_front(h, vp):
            """Per-head softmax-division front half: bf16 numerator staging
            (DVE cast copy), denominator row DMA'd straight out of PSUM,
            DMA-spread DVE reciprocal (bf16 out), DMA broadcast to a bf16
            divisor block. The multiplies are deferred (div_flush)."""
            parts = []
            for qc in range(2):
                stage = sb.tile([65, QC], f32, tag="stage", bufs=4, name="stage")
                nc.vector.tensor_copy(stage, vp[qc][0:65, :])
                scr = dramp.tile([1, QC], f32, tag="scr", bufs=4, name="scr")
                nc.sync.dma_start(scr, stage[64:65, :])
                rsp = sb.tile([DK, NB], f32, tag="rsp", bufs=4, name="rsp")
                nc.sync.dma_start(rsp, scr.rearrange("o (a b) -> a (o b)", a=DK))
                nc.vector.reciprocal(rsp, rsp)
                scr2 = dramp.tile([1, QC], f32, tag="scr2", bufs=4, name="scr2")
                nc.sync.dma_start(scr2.rearrange("o (a b) -> a (o b)", a=DK), rsp)
                bc = sb.tile([DK, QC], f32, tag="bc", bufs=4, name="bc")
                nc.sync.dma_start(bc, scr2.broadcast_to([DK, QC]))
                parts.append((stage, bc))
            div_pend.append((h, parts))

        def div_flush(keep=0):
            while len(div_pend) > keep:
                h, parts = div_pend.pop(0)
                dbq, off = h // 2, (h % 2) * DK
                for qc in range(2):
                    qsl = slice(qc * QC, (qc + 1) * QC)
                    stage, bc = parts[qc]
                    if off == 0:
                        nc.vector.tensor_mul(ot[dbq][0:DK, qsl], stage[0:DK, :], bc)
                    else:
                        tmp = sb.tile([DK, QC], bf16, tag="tmp", bufs=2, name="tmp")
                        nc.vector.tensor_mul(tmp, stage[0:DK, :], bc)
                        nc.gpsimd.dma_start(ot[dbq][DK:P, qsl], tmp)

        def gen_pv_pair(d, pts):
            """PV + softmax division for head pair (2d, 2d+1), yielded in 8
            groups of 4 matmuls. pts = (ptA_lo, ptA_hi, ptB_lo, ptB_hi)."""
            ptA_lo, ptA_hi, ptB_lo, ptB_hi = pts
            for hl, (pt_lo, pt_hi) in ((0, (ptA_lo, ptA_hi)), (1, (ptB_lo, ptB_hi))):
                h = 2 * d + hl
                # Flush the oldest pending division BEFORE this head's PV:
                # its bc-broadcast chain is then >=half a unit old, so the
                # DVE multiply never head-blocks the Vector queue.
                div_flush(keep=1)
                vp = [
                    ps.tile([P, QC], f32, tag="vp", bufs=2, name="vpacc")
                    for _ in range(2)
                ]
                for g in range(4):
                    for kb in (2 * g, 2 * g + 1):
                        ptX = pt_lo if kb < 4 else pt_hi
                        for qc in range(2):
                            nc.tensor.matmul(
                                vp[qc][0:65, :],
                                vaug[kb][:, h * 65 : (h + 1) * 65],
                                ptX[:, kb % 4, qc, :],
                                start=(kb == 0),
                                stop=(kb == NB - 1),
                            )
                    yield
                div_front(h, vp)

        # ---- scores + exp unit -----------------------------------------
        def emit_scores_unit(d, qpt, pull, rate):
            """Row-packed scores for head pair d + exp ACTs; pulls `rate`
            filler chunks from the global deque after each kb-step. Unit 0
            pulls at 4/step so the deque is empty by unit 1 (the pt-ring
            recycling safety analysis assumes pv(d-1) is pulled first and
            finishes by kb-step 4 of unit d)."""
            ptA_lo = sb.tile([P, 4, 2, QC], bf16, tag="pt", bufs=6, name="ptAlo")
            ptB_lo = sb.tile([P, 4, 2, QC], bf16, tag="pt", bufs=6, name="ptBlo")
            ptA_hi = sb.tile([P, 4, 2, QC], bf16, tag="pt", bufs=6, name="ptAhi")
            ptB_hi = sb.tile([P, 4, 2, QC], bf16, tag="pt", bufs=6, name="ptBhi")
            for kb in range(NB):
                ksl = slice(kb * P, (kb + 1) * P)
                ptA = ptA_lo if kb < 4 else ptA_hi
                ptB = ptB_lo if kb < 4 else ptB_hi
                spA = ps.tile([P, 2, QC], f32, tag="big", bufs=3, name="spA")
                spB = ps.tile([P, 2, QC], f32, tag="big", bufs=3, name="spB")
                # A-qc0, A-qc1, B-qc0, B-qc1: the order under which the PE
                # overlaps the row-tile pairs (measured ~1.6x on scores).
                for qc in range(2):
                    qsl = slice(qc * QC, (qc + 1) * QC)
                    nc.tensor.matmul(
                        spA[:, qc, :], kpt[d][0:DK, ksl], qpt[0:DK, qsl],
                        start=True, stop=True,
                    )
                for qc in range(2):
                    qsl = slice(qc * QC, (qc + 1) * QC)
                    nc.tensor.matmul(
                        spB[:, qc, :], kpt[d][DK:P, ksl], qpt[DK:P, qsl],
                        start=True, stop=True,
                    )
                nc.scalar.activation(
                    ptA[:, kb % 4, :, :], spA, AF.Exp, scale=0.03125
                )
                nc.scalar.activation(
                    ptB[:, kb % 4, :, :], spB, AF.Exp, scale=0.03125
                )
                for _ in range(rate[kb]):
                    pull()
            return ptA_lo, ptA_hi, ptB_lo, ptB_hi

        # ---- O projection ----------------------------------------------
        oaccs = [None] * NB

        def oproj_head(s, use_vp=False):
            if use_vp:
                oaccs[s] = [
                    ps.tile([P, QC], f32, tag="vp", bufs=2, name="oacc_vp")
                    for _ in range(2)
                ]
            else:
                bigacc = ps.tile([P, 2, QC], f32, tag="big", bufs=3, name="oacc")
                oaccs[s] = [bigacc[:, 0, :], bigacc[:, 1, :]]
            if with_bo:
                for c in range(2):
                    nc.tensor.matmul(
                        oaccs[s][c], onesb[0:1, 0:P],
                        bo_row[0:1, c * QC : (c + 1) * QC],
                        start=True, stop=False,
                    )

        def oproj_db(s, db):
            for c in range(2):
                nc.tensor.matmul(
                    oaccs[s][c],
                    ot[db][:, s * P : (s + 1) * P],
                    wo_h[0][:, db, c * QC : (c + 1) * QC],
                    start=(db == 0 and not with_bo),
                    stop=False,
                )

        def oproj_finish(s):
            for c in range(2):
                nc.tensor.matmul(
                    oaccs[s][c],
                    ot[NB - 1][:, s * P : (s + 1) * P],
                    wo_h[0][:, NB - 1, c * QC : (c + 1) * QC],
                    start=False, stop=True,
                )
            for c in range(2):
                o = sb.tile([P, QC], f32, tag="obuf", bufs=2, name="obuf")
                nc.scalar.activation(o, oaccs[s][c], AF.Relu)
                nc.sync.dma_start(
                    outd[s * P : (s + 1) * P, c * QC : (c + 1) * QC], o
                )
            oaccs[s] = None

        # ---- emission schedule -----------------------------------------
        # kproj(0..3) serial (ScalarE evictions; it is idle anyway);
        # kproj(4..7) deferred into unit 0's fillers so the exp stream
        # starts right after qproj(0) instead of after the whole K
        # projection. The V/WO loads recycle K xt / WK ring slots, so the
        # loader generator sits after the kproj(4..7) generators (emission
        # order derives the WAR deps); V still lands by ~45us.
        with nc.named_scope("k_proj"):
            for db in range(4):
                kproj_block(db)

        def gen_late_loads():
            vt.extend(load_transposed(Vd, (nc.sync, nc.sync)))
            wo_h.append(wload(WOd, nc.sync))
            yield

        with nc.named_scope("q0"):
            for _ in gen_qproj(0, on_scalar=True):
                pass

        fillers = deque()

        def pull():
            while fillers:
                try:
                    next(fillers[0])
                    return
                except StopIteration:
                    fillers.popleft()

        pend_pts = {}
        with nc.named_scope("attention"):
            for d in range(NB):
                if d == 0:
                    for db in range(4, NB):
                        fillers.append(gen_kproj(db))
                    fillers.append(gen_late_loads())
                    fillers.append(gen_qproj(1))
                    for sblk in range(NB):
                        fillers.append(gen_vproj(sblk))
                else:
                    fillers.append(gen_pv_pair(d - 1, pend_pts.pop(d - 1)))
                    if d + 1 < NB:
                        fillers.append(gen_qproj(d + 1))
                # Pull rates sized so pv(d-1)'s early groups are emitted
                # before kb-step 4 (pt ring recycling safety) while keeping
                # per-step filler work roughly matched to the 2.1us/step
                # exp cost.
                # (pull totals >= per-unit supply, so the deque fully
                # drains every unit and pv(d-1) always starts at pull 1)
                rate = (
                    [5, 5, 5, 5, 5, 5, 5, 5] if d == 0
                    else [2, 1, 2, 1, 2, 1, 1, 1]
                )
                pend_pts[d] = emit_scores_unit(d, qpt_slot[d], pull, rate)
            while fillers:
                pull()

        def gen_pv7(pts):
            """Pair 7 PV + division, processed qc-half-major so ot[7]'s
            first 512 columns land ~6us earlier and the O-projection
            finishes for seq-blocks 0-3 can start while the qc1 half still
            drains. Division multiplies are deferred to the very end (their
            bc chains are aged by then, so the DVE queue never blocks)."""
            ptA_lo, ptA_hi, ptB_lo, ptB_hi = pts
            halves = []
            for qc in range(2):
                for hl, (lo, hi) in ((0, (ptA_lo, ptA_hi)), (1, (ptB_lo, ptB_hi))):
                    h = 2 * (NB - 1) + hl
                    vp = ps.tile([P, QC], f32, tag="vp", bufs=2, name="vp7")
                    for g in range(4):
                        for kb in (2 * g, 2 * g + 1):
                            ptX = lo if kb < 4 else hi
                            nc.tensor.matmul(
                                vp[0:65, :],
                                vaug[kb][:, h * 65 : (h + 1) * 65],
                                ptX[:, kb % 4, qc, :],
                                start=(kb == 0),
                                stop=(kb == NB - 1),
                            )
                        yield
                    stage = sb.tile([65, QC], f32, tag="stage", bufs=4, name="st7")
                    nc.vector.tensor_copy(stage, vp[0:65, :])
                    scr = dramp.tile([1, QC], f32, tag="scr", bufs=4, name="scr")
                    nc.sync.dma_start(scr, stage[64:65, :])
                    rsp = sb.tile([DK, NB], f32, tag="rsp", bufs=4, name="rsp")
                    nc.sync.dma_start(
                        rsp, scr.rearrange("o (a b) -> a (o b)", a=DK)
                    )
                    nc.vector.reciprocal(rsp, rsp)
                    scr2 = dramp.tile([1, QC], f32, tag="scr2", bufs=4, name="scr2")
                    nc.sync.dma_start(
                        scr2.rearrange("o (a b) -> a (o b)", a=DK), rsp
                    )
                    bc = sb.tile([DK, QC], f32, tag="bc", bufs=4, name="bc")
                    nc.sync.dma_start(bc, scr2.broadcast_to([DK, QC]))
                    halves.append((h, qc, stage, bc))
                # multiplies for this qc half (both heads) - emitted after
                # the copies so the DVE queue stays unblocked
                if qc == 1:
                    continue
                for h, hqc, stage, bc in halves[:]:
                    qsl = slice(hqc * QC, (hqc + 1) * QC)
                    if h % 2 == 0:
                        nc.vector.tensor_mul(
                            ot[NB - 1][0:DK, qsl], stage[0:DK, :], bc
                        )
                    else:
                        tmp = sb.tile(
                            [DK, QC], bf16, tag="tmp", bufs=2, name="tmp"
                        )
                        nc.vector.tensor_mul(tmp, stage[0:DK, :], bc)
                        nc.gpsimd.dma_start(ot[NB - 1][DK:P, qsl], tmp)
                    halves.remove((h, hqc, stage, bc))
            for h, hqc, stage, bc in halves:
                qsl = slice(hqc * QC, (hqc + 1) * QC)
                if h % 2 == 0:
                    nc.vector.tensor_mul(
                        ot[NB - 1][0:DK, qsl], stage[0:DK, :], bc
                    )
                else:
                    tmp = sb.tile([DK, QC], bf16, tag="tmp", bufs=2, name="tmp")
                    nc.vector.tensor_mul(tmp, stage[0:DK, :], bc)
                    nc.gpsimd.dma_start(ot[NB - 1][DK:P, qsl], tmp)

        # ---- tail: PV(7) + O projection --------------------------------
        with nc.named_scope("o_proj"):
            div_flush()
            g = gen_pv7(pend_pts.pop(NB - 1))

            def prologue():
                for s in range(3):
                    oproj_head(s)
                    for db in range(NB - 1):
                        oproj_db(s, db)
                        yield

            og = prologue()
            _S = object()
            more_g = more_og = True
            while more_g or more_og:
                if more_g:
                    more_g = next(g, _S) is not _S
                if more_og:
                    more_og = next(og, _S) is not _S
                if more_og:
                    more_og = next(og, _S) is not _S
            # 4th prologue block on the vp banks (free once pair 7's last
            # PV accumulators are staged out) - fills the div(7) latency.
            oproj_head(3, use_vp=True)
            for db in range(NB - 1):
                oproj_db(3, db)
            for s in range(4, NB - 1):
                oproj_finish(s - 4)
                oproj_head(s)
                for db in range(NB - 1):
                    oproj_db(s, db)
            oproj_finish(3)
            oproj_finish(4)
            oproj_head(NB - 1)
            for db in range(NB - 1):
                oproj_db(NB - 1, db)
            oproj_finish(5)
            oproj_finish(6)
            oproj_finish(7)

    _split_wide_waits(nc)
    return nc


_NC_CACHE = {}


def kernel(Q, K, V, WQ, bQ, WK, bK, WV, bV, WO, bO, h):
    bfl = ml_dtypes.bfloat16
    Q, K, V = (np.ascontiguousarray(np.asarray(x, np.float32).astype(bfl))
               for x in (Q, K, V))
    WQ, WK, WV, WO = (
        np.ascontiguousarray(np.asarray(x, np.float32).astype(bfl))
        for x in (WQ, WK, WV, WO)
    )
    bQ, bK, bV, bO = (
        np.ascontiguousarray(np.asarray(x, np.float32)) for x in (bQ, bK, bV, bO)
    )
    h = int(np.asarray(h))
    assert h == H, f"kernel specialized for h=16, got {h}"
    B = Q.shape[0]
    assert Q.shape == (B, S, D) and B == N_CORES

    key = (
        bool(np.any(bQ)) or bool(np.any(bK)),
        bool(np.any(bV)),
        bool(np.any(bO)),
    )
    if key not in _NC_CACHE:
        _NC_CACHE[key] = _build_nc(*key)
    nc = _NC_CACHE[key]

    in_maps = [
        {
            "Q": Q[b], "K": K[b], "V": V[b],
            "WQ": WQ, "WK": WK, "WV": WV, "WO": WO,
            "bQ": bQ, "bK": bK, "bV": bV, "bO": bO,
        }
        for b in range(B)
    ]
    trace = os.environ.get("BASS_MHA_TRACE") == "1"
    res = run_bass_kernel_spmd(
        nc, in_maps, core_ids=list(range(N_CORES)), trace=trace
    )
    if trace:
        kernel.last_results = res
    return np.stack([res.results[b]["out"] for b in range(B)], axis=0)
